# revision 5
# baseline (speedup 1.0000x reference)
"""ABCNN forward kernel for one TRN2 chip (8 NeuronCores), Bass/Tile.

Strategy:
- Data-parallel over batch: 2048 items -> 256 per core.
- Embedding gather on-device via SWDGE indirect DMA (bf16 table, token-major:
  each token's 256-dim row lands on one partition).
- Compute packs 4 items on 128 partitions (4 x 32 tokens); stage-B tensors
  (seq len 34) are kept D-major (d on partitions, 4 x 34 = 136 on free axis).
- All seq-axis convolutions/poolings are folded into small banded constant
  matrices applied via TensorE matmuls (host-precomputed, kron(I4, .)).
- Euclid attention via the cdist trick: d2 = r1+r2-2*Gram, Gram on TensorE
  with bf16 operands, elementwise chain f32 on DVE/ACT.
- Final softmax over the batch dim is computed on host (trivial (2048,2) op).
"""
import sys, os
sys.path.insert(0, '/opt/trn_rl_repo')
import numpy as np
import ml_dtypes

B, L, D, V = 2048, 32, 256, 100000
L2 = L + 2          # 34
NCORES = 8
BPC = B // NCORES   # 256 items per core
NSUP = 8            # supers per core (32 items each)
NPAIR = 4           # pairs per super
bf16 = ml_dtypes.bfloat16

_graph_cache = {}


def _host_consts(inp):
    """Build all constant host arrays from the (numpy f32) inputs."""
    c1w = np.asarray(inp["c1w"], np.float32)
    c1b = np.float32(inp["c1b"])
    c2w = np.asarray(inp["c2w"], np.float32)
    c2b = np.float32(inp["c2b"])
    c3w = np.asarray(inp["c3w"], np.float32)
    c3b = np.float32(inp["c3b"])
    W0 = np.asarray(inp["W0"], np.float32)
    W1 = np.asarray(inp["W1"], np.float32)
    fw1 = np.asarray(inp["fw1"], np.float32)
    fb1 = np.asarray(inp["fb1"], np.float32)
    fw2 = np.asarray(inp["fw2"], np.float32)
    fb2 = np.asarray(inp["fb2"], np.float32)

    G_ch = np.zeros((L2, L), np.float32)
    M = np.zeros((2, L2, L), np.float32)
    for i in range(L2):
        for k in range(3):
            j = i + k - 2
            if 0 <= j < L:
                G_ch[i, j] += c1w[k]
                M[0, i, j] += c3w[0, k]
                M[1, i, j] += c3w[1, k]
    P = np.zeros((L, L2), np.float32)
    for i in range(L):
        P[i, i:i + 3] = 1.0 / 3.0
    G = (P @ G_ch).astype(np.float32)

    q = np.float32(c2w.sum() / L2)
    cnt = np.array([min(m + 1, 3, L2 - m) for m in range(L2)], np.float32)
    wv = (q * cnt / 3.0).astype(np.float32)          # (34,)
    wv4 = np.tile(wv, 4)                              # (136,)

    I4 = np.eye(4, dtype=np.float32)

    def b16(x):
        return np.ascontiguousarray(x).astype(bf16)

    C = {}
    C["cg4"] = b16(np.kron(I4, G).T)                          # (128,128)
    C["cg4m2"] = b16(np.kron(I4, G).T * -2.0)
    C["cm4r"] = b16(np.kron(I4, M[0]).T)                      # (128,136)
    C["cm4a"] = b16(np.kron(I4, M[1]).T)
    C["cm4rm2"] = b16(np.kron(I4, M[0]).T * -2.0)
    C["cm4am2"] = b16(np.kron(I4, M[1]).T * -2.0)
    C["w0w1"] = b16(np.tile(np.concatenate([W0, W1], 1), (4, 1)))  # (128,512)
    C["maskA"] = np.kron(I4, np.ones((L, L), np.float32))     # (128,128) f32
    pos_item = np.arange(4 * L2) // L2
    mB = (pos_item[:, None] == pos_item[None, :]).astype(np.float32)  # (136,136)
    mask_b = np.zeros((128, 272), np.float32)
    mask_b[:, :136] = mB[:128]
    mask_b[0:8, 136:272] = mB[128:]
    C["maskB"] = mask_b
    IW = np.diag(wv4).astype(np.float32)                      # (136,136)
    iw = np.zeros((128, 272), np.float32)
    iw[:, :136] = IW[:128]
    iw[0:8, 136:272] = IW[128:]
    C["iw"] = b16(iw)
    C["wrow"] = b16((wv4 * -0.5).reshape(1, 136))             # -0.5 compensates c2Tm2
    C["i128b"] = b16(np.eye(128, dtype=np.float32))
    C["i8f"] = np.eye(8, dtype=np.float32)
    C["onescol_b"] = b16(np.ones((128, 1), np.float32))
    C["quartcol_b"] = b16(np.full((128, 1), 0.25, np.float32))
    C["ones1x128_b"] = b16(np.ones((1, 128), np.float32))
    C["ones1x8_b"] = b16(np.ones((1, 8), np.float32))
    C["ones1x8_f"] = np.ones((1, 8), np.float32)
    C["bias4"] = np.stack([
        np.full(128, c1b, np.float32),
        np.full(128, -2.0 * c1b, np.float32),
        np.full(128, c3b, np.float32),
        np.full(128, -2.0 * c3b, np.float32),
    ], axis=1)                                                # (128,4) f32
    C["fw1r"] = np.ascontiguousarray(
        fw1.reshape(4, 128, 64).transpose(1, 0, 2))           # (128,4,64) f32
    C["fb1e"] = (fb1 + c2b * fw1.sum(0)).reshape(1, 64).astype(np.float32)
    C["fw2"] = fw2.astype(np.float32)                          # (64,2)
    C["fb2"] = fb2.reshape(1, 2).astype(np.float32)
    return C


CONST_SPECS = [
    # name, shape, dtype ("b"=bf16, "f"=f32)
    ("cg4", [128, 128], "b"), ("cg4m2", [128, 128], "b"),
    ("cm4r", [128, 136], "b"), ("cm4a", [128, 136], "b"),
    ("cm4rm2", [128, 136], "b"), ("cm4am2", [128, 136], "b"),
    ("w0w1", [128, 512], "b"),
    ("maskA", [128, 128], "f"), ("maskB", [128, 272], "f"),
    ("iw", [128, 272], "b"), ("wrow", [1, 136], "b"),
    ("i128b", [128, 128], "b"), ("i8f", [8, 8], "f"),
    ("onescol_b", [128, 1], "b"), ("quartcol_b", [128, 1], "b"),
    ("ones1x128_b", [1, 128], "b"), ("ones1x8_b", [1, 8], "b"),
    ("ones1x8_f", [1, 8], "f"),
    ("bias4", [128, 4], "f"),
    ("fw1r", [128, 4, 64], "f"), ("fb1e", [1, 64], "f"),
    ("fw2", [64, 2], "f"), ("fb2", [1, 2], "f"),
]


def build_graph():
    if "nc" in _graph_cache:
        return _graph_cache["nc"]
    import concourse.bass as bass
    import concourse.bacc as bacc
    import concourse.mybir as mybir
    from concourse.tile import TileContext

    f32 = mybir.dt.float32
    bfd = mybir.dt.bfloat16
    i32 = mybir.dt.int32
    ALU = mybir.AluOpType
    ACTF = mybir.ActivationFunctionType
    AX = mybir.AxisListType

    nc = bacc.Bacc("TRN2")
    embed_d = nc.declare_dram_parameter("embed", [V, D], bfd, isOutput=False)
    idx1_d = nc.declare_dram_parameter("idx1", [128, 64], i32, isOutput=False)
    idx2_d = nc.declare_dram_parameter("idx2", [128, 64], i32, isOutput=False)
    cparams = {}
    for name, shape, dt in CONST_SPECS:
        cparams[name] = nc.declare_dram_parameter(
            name, shape, bfd if dt == "b" else f32, isOutput=False)
    out_d = nc.declare_dram_parameter("out", [2, 256], f32, isOutput=True)

    with TileContext(nc) as tc:
        with (
            tc.tile_pool(name="consts", bufs=1) as cpool,
            tc.tile_pool(name="ebuf", bufs=2) as epool,
            tc.tile_pool(name="work", bufs=3) as wpool,
            tc.tile_pool(name="ps", bufs=4, space="PSUM") as pspool,
            tc.tile_pool(name="pssm", bufs=2, space="PSUM") as smpool,
            tc.tile_pool(name="psfc", bufs=2, space="PSUM") as fcpool,
        ):
            # ---- load constants + indices
            cs = {}
            for name, shape, dt in CONST_SPECS:
                t = cpool.tile(shape, bfd if dt == "b" else f32, tag=f"c_{name}")
                nc.sync.dma_start(out=t[:], in_=cparams[name][:])
                cs[name] = t
            idx1t = cpool.tile([128, 64], i32, tag="idx1")
            nc.sync.dma_start(out=idx1t[:], in_=idx1_d[:])
            idx2t = cpool.tile([128, 64], i32, tag="idx2")
            nc.sync.dma_start(out=idx2t[:], in_=idx2_d[:])
            logits_sb = cpool.tile([2, 256], f32, tag="logits")

            biasv = {
                "c1b": cs["bias4"][:, 0:1], "c1bm2": cs["bias4"][:, 1:2],
                "c3b": cs["bias4"][:, 2:3], "c3bm2": cs["bias4"][:, 3:4],
            }

            for s in range(NSUP):
                e1t = epool.tile([128, 8, D], bfd, tag="e1")
                e2t = epool.tile([128, 8, D], bfd, tag="e2")
                for j in range(8):
                    nc.gpsimd.indirect_dma_start(
                        out=e1t[:, j, :], out_offset=None, in_=embed_d[:],
                        in_offset=bass.IndirectOffsetOnAxis(
                            ap=idx1t[:, s * 8 + j:s * 8 + j + 1], axis=0))
                    nc.gpsimd.indirect_dma_start(
                        out=e2t[:, j, :], out_offset=None, in_=embed_d[:],
                        in_offset=bass.IndirectOffsetOnAxis(
                            ap=idx2t[:, s * 8 + j:s * 8 + j + 1], axis=0))

                for p in range(NPAIR):
                    pi = s * NPAIR + p  # global pair idx [0,32)
                    e1p = e1t[:, 2 * p:2 * p + 2, :]
                    e2p = e2t[:, 2 * p:2 * p + 2, :]

                    # ---- conv1 (token-major, both packs at once)
                    rep1ps = pspool.tile([128, 512], f32, tag="ps")
                    nc.tensor.matmul(rep1ps[:], lhsT=cs["cg4"][:],
                                     rhs=e1p.rearrange("p a b -> p (a b)"),
                                     start=True, stop=True)
                    rep1sb = wpool.tile([128, 2, D], bfd, tag="rep1")
                    nc.scalar.activation(
                        rep1sb.rearrange("p a b -> p (a b)"), rep1ps[:],
                        ACTF.Identity, bias=biasv["c1b"], scale=1.0)
                    rep2ps = pspool.tile([128, 512], f32, tag="ps")
                    nc.tensor.matmul(rep2ps[:], lhsT=cs["cg4"][:],
                                     rhs=e2p.rearrange("p a b -> p (a b)"),
                                     start=True, stop=True)
                    rep2sb = wpool.tile([128, 2, D], bfd, tag="rep2")
                    nc.scalar.activation(
                        rep2sb.rearrange("p a b -> p (a b)"), rep2ps[:],
                        ACTF.Identity, bias=biasv["c1b"], scale=1.0)

                    # ---- sigma A cols (token-major squares)
                    sqA = wpool.tile([128, 2, D], bfd, tag="sqA")
                    nc.vector.tensor_tensor(
                        out=sqA.rearrange("p a b -> p (a b)"),
                        in0=rep1sb.rearrange("p a b -> p (a b)"),
                        in1=rep1sb.rearrange("p a b -> p (a b)"), op=ALU.mult)
                    r1c = wpool.tile([128, 2], f32, tag="r1c")
                    nc.vector.reduce_sum(r1c[:], sqA[:], axis=AX.X)
                    sqA2 = wpool.tile([128, 2, D], bfd, tag="sqA2")
                    nc.vector.tensor_tensor(
                        out=sqA2.rearrange("p a b -> p (a b)"),
                        in0=rep2sb.rearrange("p a b -> p (a b)"),
                        in1=rep2sb.rearrange("p a b -> p (a b)"), op=ALU.mult)
                    r2c = wpool.tile([128, 2], f32, tag="r2c")
                    nc.vector.reduce_sum(r2c[:], sqA2[:], axis=AX.X)
                    r1cb = wpool.tile([128, 2], bfd, tag="r1cb")
                    nc.vector.tensor_copy(r1cb[:], r1c[:])

                    for a in range(2):  # packs within pair
                        g = 8 * pi + 4 * a  # first item of pack (core-local)

                        # ---- repT direct mms (+bias on copy)
                        r1Tps = pspool.tile([128, 256], f32, tag="ps")
                        for sl in range(2):
                            nc.tensor.matmul(
                                r1Tps[:, 128 * sl:128 * (sl + 1)],
                                lhsT=e1t[:, 2 * p + a, 128 * sl:128 * (sl + 1)],
                                rhs=cs["cg4"][:], start=True, stop=True)
                        rep1T = wpool.tile([128, 2, 128], bfd, tag="rep1T")
                        nc.scalar.activation(
                            rep1T.rearrange("p a b -> p (a b)"), r1Tps[:],
                            ACTF.Identity, bias=biasv["c1b"], scale=1.0)
                        r2Tps = pspool.tile([128, 256], f32, tag="ps")
                        for sl in range(2):
                            nc.tensor.matmul(
                                r2Tps[:, 128 * sl:128 * (sl + 1)],
                                lhsT=e2t[:, 2 * p + a, 128 * sl:128 * (sl + 1)],
                                rhs=cs["cg4m2"][:], start=True, stop=True)
                        rep2Tm2 = wpool.tile([128, 2, 128], bfd, tag="rep2Tm2")
                        nc.scalar.activation(
                            rep2Tm2.rearrange("p a b -> p (a b)"), r2Tps[:],
                            ACTF.Identity, bias=biasv["c1bm2"], scale=1.0)

                        # ---- r1 row (via identity mm)
                        r1rps = smpool.tile([1, 128], f32, tag="pssm")
                        nc.tensor.matmul(r1rps[:], lhsT=r1cb[:, a:a + 1],
                                         rhs=cs["i128b"][:], start=True, stop=True)
                        r1row = wpool.tile([1, 128], bfd, tag="r1row")
                        nc.vector.tensor_copy(r1row[:], r1rps[:])

                        # ---- d2T = -2 Gram + r1row (+ r2col in chain)
                        d2T = pspool.tile([128, 128], f32, tag="ps")
                        nc.tensor.matmul(d2T[:], lhsT=rep2Tm2[:, 0, :],
                                         rhs=rep1T[:, 0, :], start=True, stop=False)
                        nc.tensor.matmul(d2T[:], lhsT=rep2Tm2[:, 1, :],
                                         rhs=rep1T[:, 1, :], start=False, stop=False)
                        nc.tensor.matmul(d2T[:], lhsT=cs["ones1x128_b"][:],
                                         rhs=r1row[:], start=False, stop=True)

                        # ---- euclid chain A -> A1T (bf16, masked)
                        dA = wpool.tile([128, 128], f32, tag="dA")
                        nc.vector.tensor_scalar(
                            out=dA[:], in0=d2T[:], scalar1=r2c[:, a:a + 1],
                            scalar2=0.0, op0=ALU.add, op1=ALU.max)
                        nc.scalar.activation(dA[:], dA[:], ACTF.Sqrt)
                        nc.vector.tensor_scalar(
                            out=dA[:], in0=dA[:], scalar1=1.0, scalar2=None,
                            op0=ALU.add)
                        rA = wpool.tile([128, 128], f32, tag="rA")
                        nc.vector.reciprocal_approx_fast(out=rA[:], in_=dA[:])
                        A1T = wpool.tile([128, 128], bfd, tag="A1T")
                        nc.vector.tensor_tensor(
                            out=A1T[:], in0=cs["maskA"][:], in1=rA[:],
                            op=ALU.mult)

                        # ---- att mms
                        attps = pspool.tile([128, 512], f32, tag="ps")
                        nc.tensor.matmul(attps[:, 0:256], lhsT=A1T[:],
                                         rhs=cs["w0w1"][:, 0:256],
                                         start=True, stop=True)
                        nc.tensor.matmul(attps[:, 256:512], lhsT=A1T[:],
                                         rhs=cs["w0w1"][:, 256:512],
                                         start=True, stop=True)
                        att1sb = wpool.tile([128, 256], bfd, tag="att1")
                        nc.scalar.activation(
                            att1sb[:], attps[:, 0:256], ACTF.Identity)
                        att2sb = wpool.tile([128, 256], bfd, tag="att2")
                        nc.vector.tensor_copy(att2sb[:], attps[:, 256:512])

                        # ---- cT slabs
                        c1Tps = pspool.tile([128, 272], f32, tag="ps")
                        for sl in range(2):
                            nc.tensor.matmul(
                                c1Tps[:, 136 * sl:136 * (sl + 1)],
                                lhsT=rep1sb[:, a, 128 * sl:128 * (sl + 1)],
                                rhs=cs["cm4r"][:], start=True, stop=False)
                            nc.tensor.matmul(
                                c1Tps[:, 136 * sl:136 * (sl + 1)],
                                lhsT=att1sb[:, 128 * sl:128 * (sl + 1)],
                                rhs=cs["cm4a"][:], start=False, stop=True)
                        c1T = wpool.tile([128, 2, 136], bfd, tag="c1T")
                        nc.scalar.activation(
                            c1T.rearrange("p a b -> p (a b)"), c1Tps[:],
                            ACTF.Identity, bias=biasv["c3b"], scale=1.0)
                        c2Tps = pspool.tile([128, 272], f32, tag="ps")
                        for sl in range(2):
                            nc.tensor.matmul(
                                c2Tps[:, 136 * sl:136 * (sl + 1)],
                                lhsT=rep2sb[:, a, 128 * sl:128 * (sl + 1)],
                                rhs=cs["cm4rm2"][:], start=True, stop=False)
                            nc.tensor.matmul(
                                c2Tps[:, 136 * sl:136 * (sl + 1)],
                                lhsT=att2sb[:, 128 * sl:128 * (sl + 1)],
                                rhs=cs["cm4am2"][:], start=False, stop=True)
                        c2Tm2 = wpool.tile([128, 2, 136], bfd, tag="c2Tm2")
                        nc.scalar.activation(
                            c2Tm2.rearrange("p a b -> p (a b)"), c2Tps[:],
                            ACTF.Identity, bias=biasv["c3bm2"], scale=1.0)

                        # ---- sigma B
                        sqB1 = wpool.tile([128, 2, 136], bfd, tag="sqB1")
                        nc.vector.tensor_tensor(
                            out=sqB1.rearrange("p a b -> p (a b)"),
                            in0=c1T.rearrange("p a b -> p (a b)"),
                            in1=c1T.rearrange("p a b -> p (a b)"), op=ALU.mult)
                        sqB2 = wpool.tile([128, 2, 136], bfd, tag="sqB2")
                        nc.vector.tensor_tensor(
                            out=sqB2.rearrange("p a b -> p (a b)"),
                            in0=c2Tm2.rearrange("p a b -> p (a b)"),
                            in1=c2Tm2.rearrange("p a b -> p (a b)"), op=ALU.mult)
                        c1sqm = smpool.tile([128, 1], f32, tag="pssm")
                        for sl in range(2):
                            nc.tensor.matmul(c1sqm[:], lhsT=sqB1[:, sl, 0:128],
                                             rhs=cs["onescol_b"][:],
                                             start=(sl == 0), stop=(sl == 1))
                        c1sqt = smpool.tile([8, 1], f32, tag="pssm")
                        for sl in range(2):
                            nc.tensor.matmul(c1sqt[:], lhsT=sqB1[:, sl, 128:136],
                                             rhs=cs["onescol_b"][:],
                                             start=(sl == 0), stop=(sl == 1))
                        c1sqm_f = wpool.tile([128, 1], f32, tag="c1sqm")
                        nc.vector.tensor_copy(c1sqm_f[:], c1sqm[:])
                        c1sqt_f = wpool.tile([8, 1], f32, tag="c1sqt")
                        nc.vector.tensor_copy(c1sqt_f[:], c1sqt[:])
                        c2sqr = smpool.tile([1, 136], f32, tag="pssm")
                        for sl in range(2):
                            nc.tensor.matmul(c2sqr[:], lhsT=cs["quartcol_b"][:],
                                             rhs=sqB2[:, sl, :],
                                             start=(sl == 0), stop=(sl == 1))
                        c2sqrow = wpool.tile([1, 136], bfd, tag="c2sqrow")
                        nc.vector.tensor_copy(c2sqrow[:], c2sqr[:])

                        # ---- Gram B (M-split main/tail) + c2sq row adds
                        d2B = pspool.tile([128, 272], f32, tag="ps")
                        for sl in range(2):
                            nc.tensor.matmul(d2B[:, 0:136],
                                             lhsT=c1T[:, sl, 0:128],
                                             rhs=c2Tm2[:, sl, :],
                                             start=(sl == 0), stop=False)
                            nc.tensor.matmul(d2B[0:8, 136:272],
                                             lhsT=c1T[:, sl, 128:136],
                                             rhs=c2Tm2[:, sl, :],
                                             start=(sl == 0), stop=False)
                        nc.tensor.matmul(d2B[:, 0:136], lhsT=cs["ones1x128_b"][:],
                                         rhs=c2sqrow[:], start=False, stop=True)
                        nc.tensor.matmul(d2B[0:8, 136:272], lhsT=cs["ones1x8_b"][:],
                                         rhs=c2sqrow[:], start=False, stop=True)

                        # ---- euclid chain B -> A2m (masked bf16)
                        dB = wpool.tile([128, 272], f32, tag="dB")
                        nc.vector.tensor_scalar(
                            out=dB[:, 0:136], in0=d2B[:, 0:136],
                            scalar1=c1sqm_f[:], scalar2=0.0,
                            op0=ALU.add, op1=ALU.max)
                        nc.vector.tensor_scalar(
                            out=dB[0:8, 136:272], in0=d2B[0:8, 136:272],
                            scalar1=c1sqt_f[:], scalar2=0.0,
                            op0=ALU.add, op1=ALU.max)
                        nc.scalar.activation(dB[:, 0:136], dB[:, 0:136], ACTF.Sqrt)
                        nc.scalar.activation(dB[0:8, 136:272], dB[0:8, 136:272],
                                             ACTF.Sqrt)
                        nc.vector.tensor_scalar(
                            out=dB[:, 0:136], in0=dB[:, 0:136], scalar1=1.0,
                            scalar2=None, op0=ALU.add)
                        nc.vector.tensor_scalar(
                            out=dB[0:8, 136:272], in0=dB[0:8, 136:272],
                            scalar1=1.0, scalar2=None, op0=ALU.add)
                        rB = wpool.tile([128, 272], f32, tag="rB")
                        nc.vector.reciprocal_approx_fast(
                            out=rB[:, 0:136], in_=dB[:, 0:136])
                        nc.vector.reciprocal_approx_fast(
                            out=rB[0:8, 136:272], in_=dB[0:8, 136:272])
                        A2m = wpool.tile([128, 272], bfd, tag="A2m")
                        nc.vector.tensor_tensor(
                            out=A2m[:, 0:136], in0=cs["maskB"][:, 0:136],
                            in1=rB[:, 0:136], op=ALU.mult)
                        nc.vector.tensor_tensor(
                            out=A2m[0:8, 136:272], in0=cs["maskB"][0:8, 136:272],
                            in1=rB[0:8, 136:272], op=ALU.mult)

                        # ---- col / row
                        colm = wpool.tile([128, 1], f32, tag="colm")
                        nc.vector.reduce_sum(colm[:], A2m[:, 0:136], axis=AX.X)
                        colt = wpool.tile([8, 1], f32, tag="colt")
                        nc.vector.reduce_sum(colt[:], A2m[0:8, 136:272], axis=AX.X)
                        colmb = wpool.tile([128, 1], bfd, tag="colmb")
                        nc.vector.tensor_copy(colmb[:], colm[:])
                        coltb = wpool.tile([8, 1], bfd, tag="coltb")
                        nc.vector.tensor_copy(coltb[:], colt[:])
                        colwps = smpool.tile([1, 136], f32, tag="pssm")
                        nc.tensor.matmul(colwps[:], lhsT=colmb[:],
                                         rhs=cs["iw"][:, 0:136],
                                         start=True, stop=False)
                        nc.tensor.matmul(colwps[:], lhsT=coltb[:],
                                         rhs=cs["iw"][0:8, 136:272],
                                         start=False, stop=True)
                        colw = wpool.tile([1, 136], bfd, tag="colw")
                        nc.vector.tensor_copy(colw[:], colwps[:])
                        rowps = smpool.tile([1, 136], f32, tag="pssm")
                        nc.tensor.matmul(rowps[:], lhsT=cs["onescol_b"][:],
                                         rhs=A2m[:, 0:136], start=True, stop=False)
                        nc.tensor.matmul(rowps[:], lhsT=cs["onescol_b"][0:8, :],
                                         rhs=A2m[0:8, 136:272],
                                         start=False, stop=True)
                        roww = wpool.tile([1, 136], bfd, tag="roww")
                        nc.vector.tensor_copy(roww[:], rowps[:])
                        nc.vector.tensor_tensor(out=roww[:], in0=roww[:],
                                                in1=cs["wrow"][:], op=ALU.mult)

                        # ---- broadcast cols/rows across partitions (K=1 mms)
                        cwB = smpool.tile([128, 136], f32, tag="pssm")
                        nc.tensor.matmul(cwB[:], lhsT=cs["ones1x128_b"][:],
                                         rhs=colw[:], start=True, stop=True)
                        cwBs = wpool.tile([128, 136], bfd, tag="cwBs")
                        nc.vector.tensor_copy(cwBs[:], cwB[:])
                        rwB = smpool.tile([128, 136], f32, tag="pssm")
                        nc.tensor.matmul(rwB[:], lhsT=cs["ones1x128_b"][:],
                                         rhs=roww[:], start=True, stop=True)
                        rwBs = wpool.tile([128, 136], bfd, tag="rwBs")
                        nc.vector.tensor_copy(rwBs[:], rwB[:])

                        # ---- tmp & v reduction (v4: (128, 4 slabs, 8 pair-items))
                        if a == 0:
                            v4 = wpool.tile([128, 4, 8], f32, tag="v4")
                        tmp = wpool.tile([128, 2, 136], bfd, tag="tmp")
                        for sl in range(2):
                            nc.vector.tensor_tensor(
                                out=tmp[:, sl, :], in0=c1T[:, sl, :],
                                in1=cwBs[:], op=ALU.mult)
                        for sl in range(2):
                            nc.vector.reduce_sum(
                                v4[:, sl, 4 * a:4 * a + 4],
                                tmp[:, sl, :].rearrange("p (i j) -> p i j", j=L2),
                                axis=AX.X)
                        tmp2 = wpool.tile([128, 2, 136], bfd, tag="tmp2")
                        for sl in range(2):
                            nc.vector.tensor_tensor(
                                out=tmp2[:, sl, :], in0=c2Tm2[:, sl, :],
                                in1=rwBs[:], op=ALU.mult)
                        for sl in range(2):
                            nc.vector.reduce_sum(
                                v4[:, 2 + sl, 4 * a:4 * a + 4],
                                tmp2[:, sl, :].rearrange("p (i j) -> p i j", j=L2),
                                axis=AX.X)

                    # ---- fc head (per pair: 8 items)
                    hps = fcpool.tile([8, 64], f32, tag="psfc")
                    for k in range(4):
                        nc.tensor.matmul(hps[:], lhsT=v4[:, k, :],
                                         rhs=cs["fw1r"][:, k, :],
                                         start=(k == 0), stop=False)
                    nc.tensor.matmul(hps[:], lhsT=cs["ones1x8_f"][:],
                                     rhs=cs["fb1e"][:], start=False, stop=True)
                    h_sb = wpool.tile([8, 64], f32, tag="h")
                    nc.scalar.activation(h_sb[:], hps[:], ACTF.Tanh)
                    hTps = fcpool.tile([64, 8], f32, tag="psfc")
                    nc.tensor.matmul(hTps[:], lhsT=h_sb[:], rhs=cs["i8f"][:],
                                     start=True, stop=True)
                    hT = wpool.tile([64, 8], f32, tag="hT")
                    nc.vector.tensor_copy(hT[:], hTps[:])
                    lgps = fcpool.tile([2, 8], f32, tag="psfc")
                    nc.tensor.matmul(lgps[:], lhsT=cs["fw2"][:], rhs=hT[:],
                                     start=True, stop=False)
                    nc.tensor.matmul(lgps[:], lhsT=cs["fb2"][:],
                                     rhs=cs["ones1x8_f"][:], start=False, stop=True)
                    nc.vector.tensor_copy(logits_sb[:, 8 * pi:8 * pi + 8], lgps[:])

            nc.sync.dma_start(out=out_d[:], in_=logits_sb[:])

    nc.compile()
    _graph_cache["nc"] = nc
    return nc


def _build_idx(sent):
    """(256, 32) batch-local tokens -> (128, 64) int32 gather indices.
    idx[t, s*8+j] = sent[s*32 + 4*j + t//32, t%32]."""
    idx = np.zeros((128, 64), np.int32)
    t = np.arange(128)
    for s in range(NSUP):
        for j in range(8):
            idx[:, s * 8 + j] = sent[s * 32 + 4 * j + t // 32, t % 32]
    return idx


def kernel(**inputs):
    sys.path.insert(0, '/root/problem')
    try:
        import axon_prof_shim
        axon_prof_shim.install()
    except Exception:
        pass
    from concourse.bass_utils import run_bass_kernel_spmd

    np_in = {k: np.asarray(v) for k, v in inputs.items()}
    C = _host_consts(np_in)
    embed_bf = np_in["embed"].astype(np.float32).astype(bf16)
    s1 = np_in["sentence1"].astype(np.int64)
    s2 = np_in["sentence2"].astype(np.int64)

    nc = build_graph()
    in_maps = []
    for c in range(NCORES):
        m = {"embed": embed_bf,
             "idx1": _build_idx(s1[c * BPC:(c + 1) * BPC]),
             "idx2": _build_idx(s2[c * BPC:(c + 1) * BPC])}
        for name, shape, dt in CONST_SPECS:
            m[name] = C[name]
        in_maps.append(m)

    trace = bool(os.environ.get("KERNEL_TRACE"))
    res = run_bass_kernel_spmd(nc, in_maps, list(range(NCORES)), trace=trace)
    kernel.last_exec_ns = res.exec_time_ns
    logits = np.zeros((B, 2), np.float32)
    for c in range(NCORES):
        logits[c * BPC:(c + 1) * BPC] = np.asarray(res.results[c]["out"]).T
    mx = logits.max(axis=0, keepdims=True)
    ex = np.exp(logits - mx)
    return (ex / ex.sum(axis=0, keepdims=True)).astype(np.float32)


# revision 6
# speedup vs baseline: 1.0118x; 1.0118x over previous
"""ABCNN forward kernel for one TRN2 chip (8 NeuronCores), Bass/Tile.

Strategy:
- Data-parallel over batch: 2048 items -> 256 per core.
- Embedding gather on-device via SWDGE indirect DMA (bf16 table, token-major:
  each token's 256-dim row lands on one partition).
- Compute packs 4 items on 128 partitions (4 x 32 tokens); stage-B tensors
  (seq len 34) are kept D-major (d on partitions, 4 x 34 = 136 on free axis).
- All seq-axis convolutions/poolings are folded into small banded constant
  matrices applied via TensorE matmuls (host-precomputed, kron(I4, .)).
- Euclid attention via the cdist trick: d2 = r1+r2-2*Gram, Gram on TensorE
  with bf16 operands, elementwise chain f32 on DVE/ACT.
- Final softmax over the batch dim is computed on host (trivial (2048,2) op).
"""
import sys, os
sys.path.insert(0, '/opt/trn_rl_repo')
import numpy as np
import ml_dtypes

B, L, D, V = 2048, 32, 256, 100000
L2 = L + 2          # 34
NCORES = 8
BPC = B // NCORES   # 256 items per core
NSUP = 8            # supers per core (32 items each)
NPAIR = 4           # pairs per super
bf16 = ml_dtypes.bfloat16

_graph_cache = {}


def _host_consts(inp):
    """Build all constant host arrays from the (numpy f32) inputs."""
    c1w = np.asarray(inp["c1w"], np.float32)
    c1b = np.float32(inp["c1b"])
    c2w = np.asarray(inp["c2w"], np.float32)
    c2b = np.float32(inp["c2b"])
    c3w = np.asarray(inp["c3w"], np.float32)
    c3b = np.float32(inp["c3b"])
    W0 = np.asarray(inp["W0"], np.float32)
    W1 = np.asarray(inp["W1"], np.float32)
    fw1 = np.asarray(inp["fw1"], np.float32)
    fb1 = np.asarray(inp["fb1"], np.float32)
    fw2 = np.asarray(inp["fw2"], np.float32)
    fb2 = np.asarray(inp["fb2"], np.float32)

    G_ch = np.zeros((L2, L), np.float32)
    M = np.zeros((2, L2, L), np.float32)
    for i in range(L2):
        for k in range(3):
            j = i + k - 2
            if 0 <= j < L:
                G_ch[i, j] += c1w[k]
                M[0, i, j] += c3w[0, k]
                M[1, i, j] += c3w[1, k]
    P = np.zeros((L, L2), np.float32)
    for i in range(L):
        P[i, i:i + 3] = 1.0 / 3.0
    G = (P @ G_ch).astype(np.float32)

    q = np.float32(c2w.sum() / L2)
    cnt = np.array([min(m + 1, 3, L2 - m) for m in range(L2)], np.float32)
    wv = (q * cnt / 3.0).astype(np.float32)          # (34,)
    wv4 = np.tile(wv, 4)                              # (136,)

    I4 = np.eye(4, dtype=np.float32)

    def b16(x):
        return np.ascontiguousarray(x).astype(bf16)

    C = {}
    C["cg4"] = b16(np.kron(I4, G).T)                          # (128,128)
    C["cg4m2"] = b16(np.kron(I4, G).T * -2.0)
    C["cm4r"] = b16(np.kron(I4, M[0]).T)                      # (128,136)
    C["cm4a"] = b16(np.kron(I4, M[1]).T)
    C["cm4rm2"] = b16(np.kron(I4, M[0]).T * -2.0)
    C["cm4am2"] = b16(np.kron(I4, M[1]).T * -2.0)
    C["w0w1"] = b16(np.tile(np.concatenate([W0, W1], 1), (4, 1)))  # (128,512)
    C["maskA"] = np.kron(I4, np.ones((L, L), np.float32))     # (128,128) f32
    pos_item = np.arange(4 * L2) // L2
    mB = (pos_item[:, None] == pos_item[None, :]).astype(np.float32)  # (136,136)
    mask_b = np.zeros((128, 272), np.float32)
    mask_b[:, :136] = mB[:128]
    mask_b[0:8, 136:272] = mB[128:]
    C["maskB"] = mask_b
    IW = np.diag(wv4).astype(np.float32)                      # (136,136)
    iwm2 = np.concatenate([IW[:128], IW[:128]], 1)            # (128,272) dup halves
    iwt2 = np.concatenate([IW[128:], IW[128:]], 1)            # (8,272)
    C["iwm2"] = b16(iwm2)
    C["iwt2"] = b16(iwt2)
    C["wrow2"] = b16(np.tile(wv4 * -0.5, 2).reshape(1, 272))  # -0.5 compensates c2Tm2
    C["i128b"] = b16(np.eye(128, dtype=np.float32))
    C["i8f"] = np.eye(8, dtype=np.float32)
    C["onescol_b"] = b16(np.ones((128, 1), np.float32))
    C["quartcol_b"] = b16(np.full((128, 1), 0.25, np.float32))
    C["ones1x128_b"] = b16(np.ones((1, 128), np.float32))
    C["ones1x8_b"] = b16(np.ones((1, 8), np.float32))
    C["ones1x8_f"] = np.ones((1, 8), np.float32)
    C["bias4"] = np.stack([
        np.full(128, c1b, np.float32),
        np.full(128, -2.0 * c1b, np.float32),
        np.full(128, c3b, np.float32),
        np.full(128, -2.0 * c3b, np.float32),
    ], axis=1)                                                # (128,4) f32
    C["fw1r"] = np.ascontiguousarray(
        fw1.reshape(4, 128, 64).transpose(1, 0, 2))           # (128,4,64) f32
    C["fb1e"] = (fb1 + c2b * fw1.sum(0)).reshape(1, 64).astype(np.float32)
    C["fw2"] = fw2.astype(np.float32)                          # (64,2)
    C["fb2"] = fb2.reshape(1, 2).astype(np.float32)
    return C


CONST_SPECS = [
    # name, shape, dtype ("b"=bf16, "f"=f32)
    ("cg4", [128, 128], "b"), ("cg4m2", [128, 128], "b"),
    ("cm4r", [128, 136], "b"), ("cm4a", [128, 136], "b"),
    ("cm4rm2", [128, 136], "b"), ("cm4am2", [128, 136], "b"),
    ("w0w1", [128, 512], "b"),
    ("maskA", [128, 128], "f"), ("maskB", [128, 272], "f"),
    ("iwm2", [128, 272], "b"), ("iwt2", [8, 272], "b"),
    ("wrow2", [1, 272], "b"),
    ("i128b", [128, 128], "b"), ("i8f", [8, 8], "f"),
    ("onescol_b", [128, 1], "b"), ("quartcol_b", [128, 1], "b"),
    ("ones1x128_b", [1, 128], "b"), ("ones1x8_b", [1, 8], "b"),
    ("ones1x8_f", [1, 8], "f"),
    ("bias4", [128, 4], "f"),
    ("fw1r", [128, 4, 64], "f"), ("fb1e", [1, 64], "f"),
    ("fw2", [64, 2], "f"), ("fb2", [1, 2], "f"),
]


def build_graph():
    if "nc" in _graph_cache:
        return _graph_cache["nc"]
    import concourse.bass as bass
    import concourse.bacc as bacc
    import concourse.mybir as mybir
    from concourse.tile import TileContext

    f32 = mybir.dt.float32
    bfd = mybir.dt.bfloat16
    i32 = mybir.dt.int32
    ALU = mybir.AluOpType
    ACTF = mybir.ActivationFunctionType
    AX = mybir.AxisListType

    nc = bacc.Bacc("TRN2")
    embed_d = nc.declare_dram_parameter("embed", [V, D], bfd, isOutput=False)
    idx1_d = nc.declare_dram_parameter("idx1", [128, 64], i32, isOutput=False)
    idx2_d = nc.declare_dram_parameter("idx2", [128, 64], i32, isOutput=False)
    cparams = {}
    for name, shape, dt in CONST_SPECS:
        cparams[name] = nc.declare_dram_parameter(
            name, shape, bfd if dt == "b" else f32, isOutput=False)
    out_d = nc.declare_dram_parameter("out", [2, 256], f32, isOutput=True)

    with TileContext(nc) as tc:
        with (
            tc.tile_pool(name="consts", bufs=1) as cpool,
            tc.tile_pool(name="ebuf", bufs=2) as epool,
            tc.tile_pool(name="work", bufs=3) as wpool,
            tc.tile_pool(name="ps", bufs=4, space="PSUM") as pspool,
            tc.tile_pool(name="pssm", bufs=2, space="PSUM") as smpool,
            tc.tile_pool(name="psfc", bufs=2, space="PSUM") as fcpool,
        ):
            # ---- load constants + indices
            cs = {}
            for name, shape, dt in CONST_SPECS:
                t = cpool.tile(shape, bfd if dt == "b" else f32, tag=f"c_{name}")
                nc.sync.dma_start(out=t[:], in_=cparams[name][:])
                cs[name] = t
            idx1t = cpool.tile([128, 64], i32, tag="idx1")
            nc.sync.dma_start(out=idx1t[:], in_=idx1_d[:])
            idx2t = cpool.tile([128, 64], i32, tag="idx2")
            nc.sync.dma_start(out=idx2t[:], in_=idx2_d[:])
            logits_sb = cpool.tile([2, 256], f32, tag="logits")

            biasv = {
                "c1b": cs["bias4"][:, 0:1], "c1bm2": cs["bias4"][:, 1:2],
                "c3b": cs["bias4"][:, 2:3], "c3bm2": cs["bias4"][:, 3:4],
            }

            for s in range(NSUP):
                e1t = epool.tile([128, 8, D], bfd, tag="e1")
                e2t = epool.tile([128, 8, D], bfd, tag="e2")
                for j in range(8):
                    nc.gpsimd.indirect_dma_start(
                        out=e1t[:, j, :], out_offset=None, in_=embed_d[:],
                        in_offset=bass.IndirectOffsetOnAxis(
                            ap=idx1t[:, s * 8 + j:s * 8 + j + 1], axis=0))
                    nc.gpsimd.indirect_dma_start(
                        out=e2t[:, j, :], out_offset=None, in_=embed_d[:],
                        in_offset=bass.IndirectOffsetOnAxis(
                            ap=idx2t[:, s * 8 + j:s * 8 + j + 1], axis=0))

                for p in range(NPAIR):
                    pi = s * NPAIR + p  # global pair idx [0,32)
                    e1p = e1t[:, 2 * p:2 * p + 2, :]
                    e2p = e2t[:, 2 * p:2 * p + 2, :]

                    # ---- conv1 (token-major, both packs at once)
                    rep1ps = pspool.tile([128, 512], f32, tag="ps")
                    nc.tensor.matmul(rep1ps[:], lhsT=cs["cg4"][:],
                                     rhs=e1p.rearrange("p a b -> p (a b)"),
                                     start=True, stop=True)
                    rep1sb = wpool.tile([128, 2, D], bfd, tag="rep1")
                    nc.scalar.activation(
                        rep1sb.rearrange("p a b -> p (a b)"), rep1ps[:],
                        ACTF.Identity, bias=biasv["c1b"], scale=1.0)
                    rep2ps = pspool.tile([128, 512], f32, tag="ps")
                    nc.tensor.matmul(rep2ps[:], lhsT=cs["cg4"][:],
                                     rhs=e2p.rearrange("p a b -> p (a b)"),
                                     start=True, stop=True)
                    rep2sb = wpool.tile([128, 2, D], bfd, tag="rep2")
                    nc.scalar.activation(
                        rep2sb.rearrange("p a b -> p (a b)"), rep2ps[:],
                        ACTF.Identity, bias=biasv["c1b"], scale=1.0)

                    # ---- sigma A cols via ACT Square + accum_out
                    sqj = wpool.tile([128, D], bfd, tag="sqj")
                    r1c = wpool.tile([128, 2], f32, tag="r1c")
                    r2c = wpool.tile([128, 2], f32, tag="r2c")
                    for a_ in range(2):
                        nc.scalar.activation(sqj[:], rep1sb[:, a_, :],
                                             ACTF.Square,
                                             accum_out=r1c[:, a_:a_ + 1])
                        nc.scalar.activation(sqj[:], rep2sb[:, a_, :],
                                             ACTF.Square,
                                             accum_out=r2c[:, a_:a_ + 1])
                    r1cb = wpool.tile([128, 2], bfd, tag="r1cb")
                    nc.vector.tensor_copy(r1cb[:], r1c[:])

                    for a in range(2):  # packs within pair
                        g = 8 * pi + 4 * a  # first item of pack (core-local)

                        # ---- repT direct mms (+bias on copy)
                        r1Tps = pspool.tile([128, 256], f32, tag="ps")
                        for sl in range(2):
                            nc.tensor.matmul(
                                r1Tps[:, 128 * sl:128 * (sl + 1)],
                                lhsT=e1t[:, 2 * p + a, 128 * sl:128 * (sl + 1)],
                                rhs=cs["cg4"][:], start=True, stop=True)
                        rep1T = wpool.tile([128, 2, 128], bfd, tag="rep1T")
                        nc.scalar.activation(
                            rep1T.rearrange("p a b -> p (a b)"), r1Tps[:],
                            ACTF.Identity, bias=biasv["c1b"], scale=1.0)
                        r2Tps = pspool.tile([128, 256], f32, tag="ps")
                        for sl in range(2):
                            nc.tensor.matmul(
                                r2Tps[:, 128 * sl:128 * (sl + 1)],
                                lhsT=e2t[:, 2 * p + a, 128 * sl:128 * (sl + 1)],
                                rhs=cs["cg4m2"][:], start=True, stop=True)
                        rep2Tm2 = wpool.tile([128, 2, 128], bfd, tag="rep2Tm2")
                        nc.scalar.activation(
                            rep2Tm2.rearrange("p a b -> p (a b)"), r2Tps[:],
                            ACTF.Identity, bias=biasv["c1bm2"], scale=1.0)

                        # ---- r1 row (via identity mm)
                        r1rps = smpool.tile([1, 128], f32, tag="pssm")
                        nc.tensor.matmul(r1rps[:], lhsT=r1cb[:, a:a + 1],
                                         rhs=cs["i128b"][:], start=True, stop=True)
                        r1row = wpool.tile([1, 128], bfd, tag="r1row")
                        nc.vector.tensor_copy(r1row[:], r1rps[:])

                        # ---- d2T = -2 Gram + r1row (+ r2col in chain)
                        d2T = pspool.tile([128, 128], f32, tag="ps")
                        nc.tensor.matmul(d2T[:], lhsT=rep2Tm2[:, 0, :],
                                         rhs=rep1T[:, 0, :], start=True, stop=False)
                        nc.tensor.matmul(d2T[:], lhsT=rep2Tm2[:, 1, :],
                                         rhs=rep1T[:, 1, :], start=False, stop=False)
                        nc.tensor.matmul(d2T[:], lhsT=cs["ones1x128_b"][:],
                                         rhs=r1row[:], start=False, stop=True)

                        # ---- euclid chain A -> A1T (bf16, masked)
                        dA = wpool.tile([128, 128], f32, tag="dA")
                        nc.vector.tensor_scalar(
                            out=dA[:], in0=d2T[:], scalar1=r2c[:, a:a + 1],
                            scalar2=0.0, op0=ALU.add, op1=ALU.max)
                        nc.scalar.activation(dA[:], dA[:], ACTF.Sqrt)
                        nc.vector.tensor_scalar(
                            out=dA[:], in0=dA[:], scalar1=1.0, scalar2=None,
                            op0=ALU.add)
                        rA = wpool.tile([128, 128], f32, tag="rA")
                        nc.vector.reciprocal_approx_fast(out=rA[:], in_=dA[:])
                        A1T = wpool.tile([128, 128], bfd, tag="A1T")
                        nc.vector.tensor_tensor(
                            out=A1T[:], in0=cs["maskA"][:], in1=rA[:],
                            op=ALU.mult)

                        # ---- att mm (att1|att2 in one shot)
                        attps = pspool.tile([128, 512], f32, tag="ps")
                        nc.tensor.matmul(attps[:], lhsT=A1T[:],
                                         rhs=cs["w0w1"][:],
                                         start=True, stop=True)
                        attsb = wpool.tile([128, 512], bfd, tag="attsb")
                        nc.scalar.activation(attsb[:], attps[:], ACTF.Identity)

                        # ---- cT slabs
                        c1Tps = pspool.tile([128, 272], f32, tag="ps")
                        for sl in range(2):
                            nc.tensor.matmul(
                                c1Tps[:, 136 * sl:136 * (sl + 1)],
                                lhsT=rep1sb[:, a, 128 * sl:128 * (sl + 1)],
                                rhs=cs["cm4r"][:], start=True, stop=False)
                            nc.tensor.matmul(
                                c1Tps[:, 136 * sl:136 * (sl + 1)],
                                lhsT=attsb[:, 128 * sl:128 * (sl + 1)],
                                rhs=cs["cm4a"][:], start=False, stop=True)
                        c1T = wpool.tile([128, 2, 136], bfd, tag="c1T")
                        nc.scalar.activation(
                            c1T.rearrange("p a b -> p (a b)"), c1Tps[:],
                            ACTF.Identity, bias=biasv["c3b"], scale=1.0)
                        c2Tps = pspool.tile([128, 272], f32, tag="ps")
                        for sl in range(2):
                            nc.tensor.matmul(
                                c2Tps[:, 136 * sl:136 * (sl + 1)],
                                lhsT=rep2sb[:, a, 128 * sl:128 * (sl + 1)],
                                rhs=cs["cm4rm2"][:], start=True, stop=False)
                            nc.tensor.matmul(
                                c2Tps[:, 136 * sl:136 * (sl + 1)],
                                lhsT=attsb[:, 256 + 128 * sl:256 + 128 * (sl + 1)],
                                rhs=cs["cm4am2"][:], start=False, stop=True)
                        c2Tm2 = wpool.tile([128, 2, 136], bfd, tag="c2Tm2")
                        nc.scalar.activation(
                            c2Tm2.rearrange("p a b -> p (a b)"), c2Tps[:],
                            ACTF.Identity, bias=biasv["c3bm2"], scale=1.0)

                        # ---- sigma B
                        sqB1 = wpool.tile([128, 2, 136], bfd, tag="sqB1")
                        nc.vector.tensor_tensor(
                            out=sqB1.rearrange("p a b -> p (a b)"),
                            in0=c1T.rearrange("p a b -> p (a b)"),
                            in1=c1T.rearrange("p a b -> p (a b)"), op=ALU.mult)
                        sqB2 = wpool.tile([128, 2, 136], bfd, tag="sqB2")
                        nc.vector.tensor_tensor(
                            out=sqB2.rearrange("p a b -> p (a b)"),
                            in0=c2Tm2.rearrange("p a b -> p (a b)"),
                            in1=c2Tm2.rearrange("p a b -> p (a b)"), op=ALU.mult)
                        c1sqm = smpool.tile([128, 1], f32, tag="pssm")
                        for sl in range(2):
                            nc.tensor.matmul(c1sqm[:], lhsT=sqB1[:, sl, 0:128],
                                             rhs=cs["onescol_b"][:],
                                             start=(sl == 0), stop=(sl == 1))
                        c1sqt = smpool.tile([8, 1], f32, tag="pssm")
                        for sl in range(2):
                            nc.tensor.matmul(c1sqt[:], lhsT=sqB1[:, sl, 128:136],
                                             rhs=cs["onescol_b"][:],
                                             start=(sl == 0), stop=(sl == 1))
                        c1sqm_f = wpool.tile([128, 1], f32, tag="c1sqm")
                        nc.vector.tensor_copy(c1sqm_f[:], c1sqm[:])
                        c1sqt_f = wpool.tile([8, 1], f32, tag="c1sqt")
                        nc.vector.tensor_copy(c1sqt_f[:], c1sqt[:])
                        c2sqr = smpool.tile([1, 136], f32, tag="pssm")
                        for sl in range(2):
                            nc.tensor.matmul(c2sqr[:], lhsT=cs["quartcol_b"][:],
                                             rhs=sqB2[:, sl, :],
                                             start=(sl == 0), stop=(sl == 1))
                        c2sqrow = wpool.tile([1, 136], bfd, tag="c2sqrow")
                        nc.vector.tensor_copy(c2sqrow[:], c2sqr[:])

                        # ---- Gram B (M-split main/tail) + c2sq row adds
                        d2B = pspool.tile([128, 272], f32, tag="ps")
                        for sl in range(2):
                            nc.tensor.matmul(d2B[:, 0:136],
                                             lhsT=c1T[:, sl, 0:128],
                                             rhs=c2Tm2[:, sl, :],
                                             start=(sl == 0), stop=False)
                            nc.tensor.matmul(d2B[0:8, 136:272],
                                             lhsT=c1T[:, sl, 128:136],
                                             rhs=c2Tm2[:, sl, :],
                                             start=(sl == 0), stop=False)
                        nc.tensor.matmul(d2B[:, 0:136], lhsT=cs["ones1x128_b"][:],
                                         rhs=c2sqrow[:], start=False, stop=True)
                        nc.tensor.matmul(d2B[0:8, 136:272], lhsT=cs["ones1x8_b"][:],
                                         rhs=c2sqrow[:], start=False, stop=True)

                        # ---- euclid chain B -> A2m (masked bf16)
                        dB = wpool.tile([128, 272], f32, tag="dB")
                        nc.vector.tensor_scalar(
                            out=dB[:, 0:136], in0=d2B[:, 0:136],
                            scalar1=c1sqm_f[:], scalar2=0.0,
                            op0=ALU.add, op1=ALU.max)
                        nc.vector.tensor_scalar(
                            out=dB[0:8, 136:272], in0=d2B[0:8, 136:272],
                            scalar1=c1sqt_f[:], scalar2=0.0,
                            op0=ALU.add, op1=ALU.max)
                        nc.scalar.activation(dB[:], dB[:], ACTF.Sqrt)
                        nc.vector.tensor_scalar(
                            out=dB[:], in0=dB[:], scalar1=1.0,
                            scalar2=None, op0=ALU.add)
                        rB = wpool.tile([128, 272], f32, tag="rB")
                        nc.vector.reciprocal_approx_fast(out=rB[:], in_=dB[:])
                        A2m = wpool.tile([128, 272], bfd, tag="A2m")
                        nc.vector.tensor_tensor(
                            out=A2m[:], in0=cs["maskB"][:],
                            in1=rB[:], op=ALU.mult)

                        # ---- col / row
                        colmt = wpool.tile([128, 2], f32, tag="colmt")
                        nc.vector.reduce_sum(
                            colmt[:], A2m.rearrange("p (a b) -> p a b", b=136),
                            axis=AX.X)
                        colmtb = wpool.tile([128, 2], bfd, tag="colmtb")
                        nc.scalar.activation(colmtb[:], colmt[:], ACTF.Identity)
                        colwps = smpool.tile([1, 272], f32, tag="pssm")
                        nc.tensor.matmul(colwps[:], lhsT=colmtb[:, 0:1],
                                         rhs=cs["iwm2"][:],
                                         start=True, stop=False)
                        nc.tensor.matmul(colwps[:], lhsT=colmtb[0:8, 1:2],
                                         rhs=cs["iwt2"][:],
                                         start=False, stop=True)
                        colw = wpool.tile([1, 272], bfd, tag="colw")
                        nc.vector.tensor_copy(colw[:], colwps[:])
                        rowps = smpool.tile([1, 136], f32, tag="pssm")
                        nc.tensor.matmul(rowps[:], lhsT=cs["onescol_b"][:],
                                         rhs=A2m[:, 0:136], start=True, stop=False)
                        nc.tensor.matmul(rowps[:], lhsT=cs["onescol_b"][0:8, :],
                                         rhs=A2m[0:8, 136:272],
                                         start=False, stop=True)
                        roww = wpool.tile([1, 272], bfd, tag="roww")
                        nc.scalar.activation(roww[:, 0:136], rowps[:],
                                             ACTF.Identity)
                        nc.scalar.activation(roww[:, 136:272], rowps[:],
                                             ACTF.Identity)
                        nc.vector.tensor_tensor(out=roww[:], in0=roww[:],
                                                in1=cs["wrow2"][:], op=ALU.mult)

                        # ---- broadcast cols/rows across partitions (K=1 mms)
                        cwB = smpool.tile([128, 272], f32, tag="pssm")
                        nc.tensor.matmul(cwB[:], lhsT=cs["ones1x128_b"][:],
                                         rhs=colw[:], start=True, stop=True)
                        cwBs = wpool.tile([128, 272], bfd, tag="cwBs")
                        nc.scalar.activation(cwBs[:], cwB[:], ACTF.Identity)
                        rwB = smpool.tile([128, 272], f32, tag="pssm")
                        nc.tensor.matmul(rwB[:], lhsT=cs["ones1x128_b"][:],
                                         rhs=roww[:], start=True, stop=True)
                        rwBs = wpool.tile([128, 272], bfd, tag="rwBs")
                        nc.scalar.activation(rwBs[:], rwB[:], ACTF.Identity)

                        # ---- tmp & v reduction (v4: (128, 4 slabs, 8 pair-items))
                        if a == 0:
                            v4 = wpool.tile([128, 4, 8], f32, tag="v4")
                        tmp = wpool.tile([128, 2, 136], bfd, tag="tmp")
                        nc.vector.tensor_tensor(
                            out=tmp.rearrange("p a b -> p (a b)"),
                            in0=c1T.rearrange("p a b -> p (a b)"),
                            in1=cwBs[:], op=ALU.mult)
                        nc.vector.reduce_sum(
                            v4[:, 0:2, 4 * a:4 * a + 4],
                            tmp.rearrange("p a (i j) -> p a i j", j=L2),
                            axis=AX.X)
                        tmp2 = wpool.tile([128, 2, 136], bfd, tag="tmp2")
                        nc.vector.tensor_tensor(
                            out=tmp2.rearrange("p a b -> p (a b)"),
                            in0=c2Tm2.rearrange("p a b -> p (a b)"),
                            in1=rwBs[:], op=ALU.mult)
                        nc.vector.reduce_sum(
                            v4[:, 2:4, 4 * a:4 * a + 4],
                            tmp2.rearrange("p a (i j) -> p a i j", j=L2),
                            axis=AX.X)

                    # ---- fc head (per pair: 8 items)
                    hps = fcpool.tile([8, 64], f32, tag="psfc")
                    for k in range(4):
                        nc.tensor.matmul(hps[:], lhsT=v4[:, k, :],
                                         rhs=cs["fw1r"][:, k, :],
                                         start=(k == 0), stop=False)
                    nc.tensor.matmul(hps[:], lhsT=cs["ones1x8_f"][:],
                                     rhs=cs["fb1e"][:], start=False, stop=True)
                    h_sb = wpool.tile([8, 64], f32, tag="h")
                    nc.scalar.activation(h_sb[:], hps[:], ACTF.Tanh)
                    hTps = fcpool.tile([64, 8], f32, tag="psfc")
                    nc.tensor.matmul(hTps[:], lhsT=h_sb[:], rhs=cs["i8f"][:],
                                     start=True, stop=True)
                    hT = wpool.tile([64, 8], f32, tag="hT")
                    nc.vector.tensor_copy(hT[:], hTps[:])
                    lgps = fcpool.tile([2, 8], f32, tag="psfc")
                    nc.tensor.matmul(lgps[:], lhsT=cs["fw2"][:], rhs=hT[:],
                                     start=True, stop=False)
                    nc.tensor.matmul(lgps[:], lhsT=cs["fb2"][:],
                                     rhs=cs["ones1x8_f"][:], start=False, stop=True)
                    nc.vector.tensor_copy(logits_sb[:, 8 * pi:8 * pi + 8], lgps[:])

            nc.sync.dma_start(out=out_d[:], in_=logits_sb[:])

    nc.compile()
    _graph_cache["nc"] = nc
    return nc


def _build_idx(sent):
    """(256, 32) batch-local tokens -> (128, 64) int32 gather indices.
    idx[t, s*8+j] = sent[s*32 + 4*j + t//32, t%32]."""
    idx = np.zeros((128, 64), np.int32)
    t = np.arange(128)
    for s in range(NSUP):
        for j in range(8):
            idx[:, s * 8 + j] = sent[s * 32 + 4 * j + t // 32, t % 32]
    return idx


def kernel(**inputs):
    sys.path.insert(0, '/root/problem')
    try:
        import axon_prof_shim
        axon_prof_shim.install()
    except Exception:
        pass
    from concourse.bass_utils import run_bass_kernel_spmd

    np_in = {k: np.asarray(v) for k, v in inputs.items()}
    C = _host_consts(np_in)
    embed_bf = np_in["embed"].astype(np.float32).astype(bf16)
    s1 = np_in["sentence1"].astype(np.int64)
    s2 = np_in["sentence2"].astype(np.int64)

    nc = build_graph()
    in_maps = []
    for c in range(NCORES):
        m = {"embed": embed_bf,
             "idx1": _build_idx(s1[c * BPC:(c + 1) * BPC]),
             "idx2": _build_idx(s2[c * BPC:(c + 1) * BPC])}
        for name, shape, dt in CONST_SPECS:
            m[name] = C[name]
        in_maps.append(m)

    trace = bool(os.environ.get("KERNEL_TRACE"))
    res = run_bass_kernel_spmd(nc, in_maps, list(range(NCORES)), trace=trace)
    kernel.last_exec_ns = res.exec_time_ns
    logits = np.zeros((B, 2), np.float32)
    for c in range(NCORES):
        logits[c * BPC:(c + 1) * BPC] = np.asarray(res.results[c]["out"]).T
    mx = logits.max(axis=0, keepdims=True)
    ex = np.exp(logits - mx)
    return (ex / ex.sum(axis=0, keepdims=True)).astype(np.float32)
